# revision 13
# baseline (speedup 1.0000x reference)
"""KWinnersTakeAll top-k mask (K=410 per row of 8192, relu+mask) for TRN2.

Device finds, per 128-row tile (rows in partitions), each row's 16-bit-prefix
bucket b of the K-th largest value, then emits a ternary mask plus a tiny
per-row side channel; the host resolves only the ~6-7 boundary-bucket
elements per row exactly.

  1. h16 = top-16 bits of each f32 (ACT-engine strided u16 copy). As int16,
     positive floats map to [0, 32767] order-isomorphically.
  2. Newton + bisect on the DVE locates b = the largest 16-bit value with
     G(b) = #{h16 >= b} >= K: one count at V0, a quadratic bucket estimate,
     then a [4,2,1] bisect from round(bhat)-4. Each count is a single i16
     tensor_scalar + accum pass (4x DVE mode, ~2.2us).
  3. ACT emits mask = Sign(x - edge(b+1)) as u8: 1 = definite winner
     (x > edge), 0 = x == edge exactly (also a winner), 255 = x < edge.
     Exactly-rounded f32 subtract makes the 0/1/255 classes exact.
  4. Per-row b values collect into a [128, 8] tile, DMA'd out once.
  5. Host: winners = (mask != 255); m = K - winners_per_row; candidates =
     elements whose top-16 bits == b; add the top-m candidates by exact f32
     value (ties -> lower index, matching the reference). Rows where the
     bracket missed (Newton tail) fall back to an exact host recompute.

Per-tile engine budget: DMA 14.6us (in 11.7 + out 2.9, the bottleneck),
ACT ~14us (h16 copy + Sign mask), DVE ~10us (4 count passes + smalls),
so the kernel runs at the per-core HBM roofline.

The module is post-processed by split_multi_waits: this toolchain's walrus
allows a single sync wait per TPB instruction, so extra semaphore waits are
hoisted onto same-engine NoOps.

Sharding: pure data parallel, 1024 rows per core across 8 cores.
"""

import numpy as np

import concourse.bass as bass
import concourse.mybir as mybir
import concourse.tile as tile
from concourse.bass_utils import run_bass_kernel_spmd


F32 = mybir.dt.float32
U16 = mybir.dt.uint16
I16 = mybir.dt.int16
I32 = mybir.dt.int32
U8 = mybir.dt.uint8
Alu = mybir.AluOpType
Act = mybir.ActivationFunctionType


def split_multi_waits(nc, max_waits: int = 1) -> int:
    """Rewrite every instruction carrying more than ``max_waits`` sem waits.
    Returns the number of instructions split."""
    n_split = 0
    fn = nc.m.functions[0]
    for bb in fn.blocks:
        insts = list(bb.instructions)
        out = []
        changed = False
        for inst in insts:
            si = inst.sync_info
            waits = list(si.on_wait) if si else []
            if len(waits) > max_waits:
                n_split += 1
                changed = True
                for i, w in enumerate(waits[:-max_waits]):
                    nop = mybir.InstNoOp(
                        name=f"{inst.name}-waitsplit-{i}", ins=[], outs=[]
                    )
                    nop.engine = inst.engine
                    nop.sync_info = mybir.SyncInfo(on_wait=[w], on_update=[])
                    out.append(nop)
                inst.sync_info = mybir.SyncInfo(
                    on_wait=waits[-max_waits:], on_update=list(si.on_update)
                )
            out.append(inst)
        if changed:
            bb.instructions = out
    return n_split


B_FULL = 8192
E = 8192
N_CORES = 8
B_CORE = B_FULL // N_CORES
K = 410                  # ceil(0.05 * 8192)
# Newton start: one count at V0, then quadratic bucket estimate and a short
# [4,2,1] bisect from bhat-4. Estimator fit on N(0,1) rows; errors beyond
# [-4,+3] fall to the host guard.
V0 = 16338               # hi16 of ~1.645 (the asymptotic 95% quantile)
NEWT_C0 = 16337.5789     # bhat = C0 + C1*d + C2*d^2, d = count(h16>=V0) - K
NEWT_C1 = 0.14814322
NEWT_C2 = -4.5165e-05
NEWT_STEPS = [4, 2, 1]
NEWT_BACK = 4.0          # bisect lo = round(bhat) - NEWT_BACK


def build_kwta(tc, out_ap, brow_ap, in_ap, b_rows):
    nc = tc.nc
    n_tiles = b_rows // 128

    with (
        tc.tile_pool(name="pio", bufs=3) as pio,
        tc.tile_pool(name="pmask", bufs=1) as pmask,
        tc.tile_pool(name="ph", bufs=2) as ph,
        tc.tile_pool(name="ppred", bufs=1) as ppred,
        tc.tile_pool(name="psmall", bufs=4) as psmall,
        tc.tile_pool(name="pbrow", bufs=1) as pbrow,
    ):
        # one half-width scratch for every count pass's elementwise output
        # (only the accum matters); DVE executes in order so reuse is
        # hazard-free. Counts run as two half-row passes whose accums add.
        H = E // 2
        pred = ppred.tile([128, H], I16)
        ball = pbrow.tile([128, n_tiles], F32)
        masks = []
        Q = E // 4

        def count_ge(h16, scalar, tag):
            """cnt[p] = #{h16[p, :] >= scalar} via two half-row 4x passes."""
            ca = psmall.tile([128, 1], F32, tag=f"{tag}_a", name=f"{tag}_a")
            cb = psmall.tile([128, 1], F32, tag=f"{tag}_b", name=f"{tag}_b")
            nc.vector.tensor_scalar(
                pred[:], h16[:, :H].bitcast(I16), scalar, 0.0, Alu.is_ge,
                Alu.add, accum_out=ca[:],
            )
            nc.vector.tensor_scalar(
                pred[:], h16[:, H:].bitcast(I16), scalar, 0.0, Alu.is_ge,
                Alu.add, accum_out=cb[:],
            )
            cnt = psmall.tile([128, 1], F32, tag=f"{tag}_s", name=f"{tag}_s")
            nc.vector.tensor_tensor(cnt[:], ca[:], cb[:], Alu.add)
            return cnt

        for ti in range(n_tiles):
            rows = slice(ti * 128, (ti + 1) * 128)

            xt = pio.tile([128, E], F32, tag="xt")
            h16 = ph.tile([128, E], U16, tag="h16")
            xu = xt[:].bitcast(U16).rearrange("p (n two) -> p n two", two=2)
            # quarter-split the load so the DVE h16 copy streams behind the
            # DMA instead of waiting for the full 4 MiB transfer
            for qi in range(4):
                nc.sync.dma_start(
                    xt[:, qi * Q:(qi + 1) * Q], in_ap[rows, qi * Q:(qi + 1) * Q]
                )
                # h16 extraction rides the otherwise-idle GPSIMD engine,
                # streaming a quarter behind the DMA. The last tile's copy
                # runs on the (by then idle) DVE instead: it is on the
                # kernel's tail critical path and DVE quarters are 2.6x
                # faster, so the final bisect starts ~2us sooner.
                eng = nc.vector if ti == n_tiles - 1 else nc.gpsimd
                eng.tensor_copy(
                    h16[:, qi * Q:(qi + 1) * Q], xu[:, qi * Q:(qi + 1) * Q, 1:2]
                )

            # --- Newton start: one count at V0, quadratic bucket estimate --
            # NB: accum_out only accumulates with op1=add (op1=mult silently
            # yields 0 on HW).
            cnt0 = count_ge(h16, float(V0), "cnt0")
            dd = psmall.tile([128, 1], F32, tag="dd")
            nc.vector.tensor_scalar(dd[:], cnt0[:], -float(K), None, Alu.add)
            t1 = psmall.tile([128, 1], F32, tag="t1")
            nc.vector.tensor_scalar(t1[:], dd[:], NEWT_C2, NEWT_C1, Alu.mult, Alu.add)
            t2 = psmall.tile([128, 1], F32, tag="t2")
            nc.vector.tensor_tensor(t2[:], t1[:], dd[:], Alu.mult)
            ri = psmall.tile([128, 1], I32, tag="ri")  # round(bhat)
            nc.vector.tensor_scalar(ri[:], t2[:], NEWT_C0, None, Alu.add)
            vf = psmall.tile([128, 1], F32, tag="vf_e")
            nc.vector.tensor_scalar(vf[:], ri[:], -NEWT_BACK, None, Alu.add)

            # --- bisect: largest b in [lo, lo+8) with G(b) >= K ------------
            for it, s in enumerate(NEWT_STEPS):
                vtest = psmall.tile([128, 1], F32, tag="vtest")
                nc.vector.tensor_scalar(vtest[:], vf[:], float(s), None, Alu.add)
                cnt = count_ge(h16, vtest[:], f"cnt{it % 2}")
                # lo += s if cnt >= K
                ges = psmall.tile([128, 1], F32, tag="ges")
                nc.vector.tensor_scalar(
                    ges[:], cnt[:], float(K) - 0.5, float(s), Alu.is_ge, Alu.mult
                )
                vf2 = psmall.tile(
                    [128, 1], F32, tag=("vf_o" if it % 2 == 0 else "vf_e")
                )
                nc.vector.tensor_tensor(vf2[:], ges[:], vf[:], Alu.add)
                vf = vf2

            # record b for the host side channel
            nc.vector.tensor_copy(ball[:, ti:ti + 1], vf[:])

            # --- ternary mask: Sign(x - edge(b+1)) -------------------------
            # edge(b+1) = bitcast((b+1) << 16); (b+1)*65536 is exact in f32
            # (14 significant bits + 16 trailing zeros), converted to i32.
            vb1 = psmall.tile([128, 1], F32, tag="vb1")
            nc.vector.tensor_scalar(vb1[:], vf[:], 1.0, None, Alu.add)
            ei = psmall.tile([128, 1], I32, tag="ei")
            nc.vector.tensor_scalar(ei[:], vb1[:], 65536.0, None, Alu.mult)
            nedge = psmall.tile([128, 1], F32, tag="nedge")
            nc.vector.tensor_scalar(
                nedge[:], ei[:].bitcast(F32), -1.0, None, Alu.mult
            )
            # Sign is Sterbenz-exact near the edge: +1 = winner, 0 = x ==
            # edge exactly (still a winner; host classifies 0 as winner),
            # 255 (= -1 as u8) = below the edge.
            mask = pmask.tile([128, E], U8, tag=f"mask{ti}")
            masks.append((mask, rows, nedge, xt))
            if ti < n_tiles - 1:
                nc.scalar.activation(mask[:], xt[:], Act.Sign, bias=nedge[:])

        # Output DMAs are deferred behind all input loads so the DMA engines
        # stream the 32 MiB of loads back-to-back; the ~23 us of mask stores
        # then overlap the last tiles' compute, and the last tile's mask
        # lands just as the store stream drains.
        for ti in range(n_tiles - 1):
            m_, r_, _, _ = masks[ti]
            nc.sync.dma_start(out_ap[r_, :], m_[:])
        nc.sync.dma_start(brow_ap[:, :], ball[:])
        mask, rows, nedge, xt = masks[n_tiles - 1]
        for qi in range(4):
            sl = slice(qi * Q, (qi + 1) * Q)
            nc.scalar.activation(mask[:, sl], xt[:, sl], Act.Sign, bias=nedge[:])
            nc.sync.dma_start(out_ap[rows, sl], mask[:, sl])


def _build_module(b_rows):
    nc = bass.Bass("TRN2", target_bir_lowering=False, debug=False)
    x = nc.dram_tensor("x", [b_rows, E], F32, kind="ExternalInput")
    out = nc.dram_tensor("out", [b_rows, E], U8, kind="ExternalOutput")
    brow = nc.dram_tensor("brow", [128, b_rows // 128], F32, kind="ExternalOutput")
    with tile.TileContext(nc) as tc:
        build_kwta(tc, out.ap(), brow.ap(), x.ap(), b_rows)
    split_multi_waits(nc)
    return nc


_NC_CACHE = {}


def _get_nc(b_rows):
    if b_rows not in _NC_CACHE:
        _NC_CACHE[b_rows] = _build_module(b_rows)
    return _NC_CACHE[b_rows]


def _host_row_fix(xrow):
    h = np.maximum(xrow, 0.0)
    idx = np.argsort(-h, kind="stable")[:K]
    mask = np.zeros(E, dtype=bool)
    mask[idx] = True
    mask &= xrow > 0
    mask[idx[0]] = True
    return mask


def _host_full(x):
    h = np.maximum(x, 0.0)
    part = np.partition(h, E - K, axis=1)
    t = part[:, E - K:E - K + 1]
    out = (h >= t).astype(np.float32)
    bad = np.flatnonzero(out.sum(axis=1) != float(K))
    for r in bad:
        out[r] = _host_row_fix(x[r]).astype(np.float32)
    return out


def _resolve_boundary(x, vmask, b):
    """Complete the device's ternary mask into the exact top-K mask.

    x: [B, E] f32. vmask: [B, E] u8 Sign output (1 / 0 = winner, 255 =
    below edge(b+1)). b: [B] bucket of the K-th value per row.
    """
    B = x.shape[0]
    win = vmask != 255
    nw = win.sum(axis=1).astype(np.int64)
    m = K - nw

    h16 = (x.view(np.uint32) >> 16).astype(np.int32)
    cand = (h16 == b[:, None].astype(np.int32)) & ~win
    ncand = cand.sum(axis=1).astype(np.int64)

    ok = (m >= 0) & (m <= ncand)
    rows_fix = np.flatnonzero(~ok)

    sel = np.flatnonzero(ok & (m > 0))
    if sel.size:
        # pad each selected row's candidates into a fixed-width matrix and
        # rank with a stable descending sort (ties -> lower column index)
        crows, ccols = np.nonzero(cand[sel])
        counts = ncand[sel]
        maxc = int(counts.max())
        starts = np.concatenate([[0], np.cumsum(counts)[:-1]])
        slot = np.arange(crows.size) - starts[crows]
        padv = np.full((sel.size, maxc), -np.inf, dtype=np.float32)
        padc = np.zeros((sel.size, maxc), dtype=np.int64)
        padv[crows, slot] = x[sel[crows], ccols]
        padc[crows, slot] = ccols
        order = np.argsort(-padv, axis=1, kind="stable")
        take = np.arange(maxc)[None, :] < m[sel][:, None]
        prow, pslot = np.nonzero(take)
        chosen_col = padc[prow, order[prow, pslot]]
        win[sel[prow], chosen_col] = True

    out = win
    # exactness guard: any row without exactly K winners gets the exact path
    bad = np.flatnonzero(out.sum(axis=1) != K)
    for r in np.union1d(rows_fix, bad):
        out[r] = _host_row_fix(x[r])
    return out.astype(np.float32)


def kernel(x: np.ndarray) -> np.ndarray:
    assert x.shape == (B_FULL, E) and x.dtype == np.float32
    try:
        nc = _get_nc(B_CORE)
    except Exception:
        return _host_full(x)
    in_maps = [
        {"x": np.ascontiguousarray(x[i * B_CORE:(i + 1) * B_CORE])}
        for i in range(N_CORES)
    ]
    try:
        res = run_bass_kernel_spmd(nc, in_maps, list(range(N_CORES)))
        vmask = np.concatenate(
            [np.asarray(r["out"]) for r in res.results], axis=0
        )
        # brow[p, ti] holds bdfor core row ti*128 + p
        b = np.concatenate(
            [np.asarray(r["brow"]).T.reshape(-1) for r in res.results], axis=0
        )
    except Exception:
        return _host_full(x)
    return _resolve_boundary(x, vmask, b)


# revision 14
# speedup vs baseline: 1.0244x; 1.0244x over previous
"""KWinnersTakeAll top-k mask (K=410 per row of 8192, relu+mask) for TRN2.

Device finds, per 128-row tile (rows in partitions), each row's 16-bit-prefix
bucket b of the K-th largest value, then emits a ternary mask plus a tiny
per-row side channel; the host resolves only the ~6-7 boundary-bucket
elements per row exactly.

  1. h16 = top-16 bits of each f32 (ACT-engine strided u16 copy). As int16,
     positive floats map to [0, 32767] order-isomorphically.
  2. Newton + bisect on the DVE locates b = the largest 16-bit value with
     G(b) = #{h16 >= b} >= K: one count at V0, a quadratic bucket estimate,
     then a [4,2,1] bisect from round(bhat)-4. Each count is a single i16
     tensor_scalar + accum pass (4x DVE mode, ~2.2us).
  3. ACT emits mask = Sign(x - edge(b+1)) as u8: 1 = definite winner
     (x > edge), 0 = x == edge exactly (also a winner), 255 = x < edge.
     Exactly-rounded f32 subtract makes the 0/1/255 classes exact.
  4. Per-row b values collect into a [128, 8] tile, DMA'd out once.
  5. Host: winners = (mask != 255); m = K - winners_per_row; candidates =
     elements whose top-16 bits == b; add the top-m candidates by exact f32
     value (ties -> lower index, matching the reference). Rows where the
     bracket missed (Newton tail) fall back to an exact host recompute.

Per-tile engine budget: DMA 14.6us (in 11.7 + out 2.9, the bottleneck),
ACT ~14us (h16 copy + Sign mask), DVE ~10us (4 count passes + smalls),
so the kernel runs at the per-core HBM roofline.

The module is post-processed by split_multi_waits: this toolchain's walrus
allows a single sync wait per TPB instruction, so extra semaphore waits are
hoisted onto same-engine NoOps.

Sharding: pure data parallel, 1024 rows per core across 8 cores.
"""

import numpy as np

import concourse.bass as bass
import concourse.mybir as mybir
import concourse.tile as tile
from concourse.bass_utils import run_bass_kernel_spmd


F32 = mybir.dt.float32
U16 = mybir.dt.uint16
I16 = mybir.dt.int16
I32 = mybir.dt.int32
U8 = mybir.dt.uint8
Alu = mybir.AluOpType
Act = mybir.ActivationFunctionType


def split_multi_waits(nc, max_waits: int = 1) -> int:
    """Rewrite every instruction carrying more than ``max_waits`` sem waits.
    Returns the number of instructions split."""
    n_split = 0
    fn = nc.m.functions[0]
    for bb in fn.blocks:
        insts = list(bb.instructions)
        out = []
        changed = False
        for inst in insts:
            si = inst.sync_info
            waits = list(si.on_wait) if si else []
            if len(waits) > max_waits:
                n_split += 1
                changed = True
                for i, w in enumerate(waits[:-max_waits]):
                    nop = mybir.InstNoOp(
                        name=f"{inst.name}-waitsplit-{i}", ins=[], outs=[]
                    )
                    nop.engine = inst.engine
                    nop.sync_info = mybir.SyncInfo(on_wait=[w], on_update=[])
                    out.append(nop)
                inst.sync_info = mybir.SyncInfo(
                    on_wait=waits[-max_waits:], on_update=list(si.on_update)
                )
            out.append(inst)
        if changed:
            bb.instructions = out
    return n_split


B_FULL = 8192
E = 8192
N_CORES = 8
B_CORE = B_FULL // N_CORES
K = 410                  # ceil(0.05 * 8192)
# Newton start: one count at V0, then quadratic bucket estimate and a short
# [4,2,1] bisect from bhat-4. Estimator fit on N(0,1) rows; errors beyond
# [-4,+3] fall to the host guard.
V0 = 16338               # hi16 of ~1.645 (the asymptotic 95% quantile)
NEWT_C0 = 16337.5789     # bhat = C0 + C1*d + C2*d^2, d = count(h16>=V0) - K
NEWT_C1 = 0.14814322
NEWT_C2 = -4.5165e-05
NEWT_STEPS = [4, 2, 1]
NEWT_BACK = 4.0          # bisect lo = round(bhat) - NEWT_BACK


def build_kwta(tc, out_ap, brow_ap, in_ap, b_rows):
    nc = tc.nc
    n_tiles = b_rows // 128

    with (
        tc.tile_pool(name="pio", bufs=3) as pio,
        tc.tile_pool(name="pmask", bufs=1) as pmask,
        tc.tile_pool(name="ph", bufs=2) as ph,
        tc.tile_pool(name="ppred", bufs=1) as ppred,
        tc.tile_pool(name="psmall", bufs=4) as psmall,
        tc.tile_pool(name="pbrow", bufs=1) as pbrow,
    ):
        # one half-width scratch for every count pass's elementwise output
        # (only the accum matters); DVE executes in order so reuse is
        # hazard-free. Counts run as two half-row passes whose accums add.
        H = E // 2
        pred = ppred.tile([128, H], I16)
        ball = pbrow.tile([128, n_tiles], F32)
        masks = []
        Q = E // 4

        def count_ge(h16, scalar, tag):
            """cnt[p] = #{h16[p, :] >= scalar} via two half-row 4x passes."""
            ca = psmall.tile([128, 1], F32, tag=f"{tag}_a", name=f"{tag}_a")
            cb = psmall.tile([128, 1], F32, tag=f"{tag}_b", name=f"{tag}_b")
            nc.vector.tensor_scalar(
                pred[:], h16[:, :H].bitcast(I16), scalar, 0.0, Alu.is_ge,
                Alu.add, accum_out=ca[:],
            )
            nc.vector.tensor_scalar(
                pred[:], h16[:, H:].bitcast(I16), scalar, 0.0, Alu.is_ge,
                Alu.add, accum_out=cb[:],
            )
            cnt = psmall.tile([128, 1], F32, tag=f"{tag}_s", name=f"{tag}_s")
            nc.vector.tensor_tensor(cnt[:], ca[:], cb[:], Alu.add)
            return cnt

        for ti in range(n_tiles):
            rows = slice(ti * 128, (ti + 1) * 128)

            xt = pio.tile([128, E], F32, tag="xt")
            h16 = ph.tile([128, E], U16, tag="h16")
            xu = xt[:].bitcast(U16).rearrange("p (n two) -> p n two", two=2)
            # quarter-split the load so the DVE h16 copy streams behind the
            # DMA instead of waiting for the full 4 MiB transfer
            for qi in range(4):
                nc.sync.dma_start(
                    xt[:, qi * Q:(qi + 1) * Q], in_ap[rows, qi * Q:(qi + 1) * Q]
                )
                # h16 extraction rides the otherwise-idle GPSIMD engine,
                # streaming a quarter behind the DMA
                nc.gpsimd.tensor_copy(
                    h16[:, qi * Q:(qi + 1) * Q], xu[:, qi * Q:(qi + 1) * Q, 1:2]
                )

            # --- Newton start: one count at V0, quadratic bucket estimate --
            # NB: accum_out only accumulates with op1=add (op1=mult silently
            # yields 0 on HW).
            cnt0 = count_ge(h16, float(V0), "cnt0")
            dd = psmall.tile([128, 1], F32, tag="dd")
            nc.vector.tensor_scalar(dd[:], cnt0[:], -float(K), None, Alu.add)
            t1 = psmall.tile([128, 1], F32, tag="t1")
            nc.vector.tensor_scalar(t1[:], dd[:], NEWT_C2, NEWT_C1, Alu.mult, Alu.add)
            t2 = psmall.tile([128, 1], F32, tag="t2")
            nc.vector.tensor_tensor(t2[:], t1[:], dd[:], Alu.mult)
            ri = psmall.tile([128, 1], I32, tag="ri")  # round(bhat)
            nc.vector.tensor_scalar(ri[:], t2[:], NEWT_C0, None, Alu.add)
            vf = psmall.tile([128, 1], F32, tag="vf_e")
            nc.vector.tensor_scalar(vf[:], ri[:], -NEWT_BACK, None, Alu.add)

            # --- bisect: largest b in [lo, lo+8) with G(b) >= K ------------
            for it, s in enumerate(NEWT_STEPS):
                vtest = psmall.tile([128, 1], F32, tag="vtest")
                nc.vector.tensor_scalar(vtest[:], vf[:], float(s), None, Alu.add)
                cnt = count_ge(h16, vtest[:], f"cnt{it % 2}")
                # lo += s if cnt >= K
                ges = psmall.tile([128, 1], F32, tag="ges")
                nc.vector.tensor_scalar(
                    ges[:], cnt[:], float(K) - 0.5, float(s), Alu.is_ge, Alu.mult
                )
                vf2 = psmall.tile(
                    [128, 1], F32, tag=("vf_o" if it % 2 == 0 else "vf_e")
                )
                nc.vector.tensor_tensor(vf2[:], ges[:], vf[:], Alu.add)
                vf = vf2

            # record b for the host side channel
            nc.vector.tensor_copy(ball[:, ti:ti + 1], vf[:])

            # --- ternary mask: Sign(x - edge(b+1)) -------------------------
            # edge(b+1) = bitcast((b+1) << 16); (b+1)*65536 is exact in f32
            # (14 significant bits + 16 trailing zeros), converted to i32.
            vb1 = psmall.tile([128, 1], F32, tag="vb1")
            nc.vector.tensor_scalar(vb1[:], vf[:], 1.0, None, Alu.add)
            ei = psmall.tile([128, 1], I32, tag="ei")
            nc.vector.tensor_scalar(ei[:], vb1[:], 65536.0, None, Alu.mult)
            nedge = psmall.tile([128, 1], F32, tag="nedge")
            nc.vector.tensor_scalar(
                nedge[:], ei[:].bitcast(F32), -1.0, None, Alu.mult
            )
            # Sign is Sterbenz-exact near the edge: +1 = winner, 0 = x ==
            # edge exactly (still a winner; host classifies 0 as winner),
            # 255 (= -1 as u8) = below the edge.
            mask = pmask.tile([128, E], U8, tag=f"mask{ti}")
            masks.append((mask, rows, nedge, xt))
            if ti < n_tiles - 1:
                nc.scalar.activation(mask[:], xt[:], Act.Sign, bias=nedge[:])

        # Output DMAs are deferred behind all input loads so the DMA engines
        # stream the 32 MiB of loads back-to-back; the ~23 us of mask stores
        # then overlap the last tiles' compute, and the last tile's mask
        # lands just as the store stream drains.
        for ti in range(n_tiles - 1):
            m_, r_, _, _ = masks[ti]
            nc.sync.dma_start(out_ap[r_, :], m_[:])
        nc.sync.dma_start(brow_ap[:, :], ball[:])
        mask, rows, nedge, xt = masks[n_tiles - 1]
        for qi in range(4):
            sl = slice(qi * Q, (qi + 1) * Q)
            nc.scalar.activation(mask[:, sl], xt[:, sl], Act.Sign, bias=nedge[:])
            nc.sync.dma_start(out_ap[rows, sl], mask[:, sl])


def _build_module(b_rows):
    nc = bass.Bass("TRN2", target_bir_lowering=False, debug=False)
    x = nc.dram_tensor("x", [b_rows, E], F32, kind="ExternalInput")
    out = nc.dram_tensor("out", [b_rows, E], U8, kind="ExternalOutput")
    brow = nc.dram_tensor("brow", [128, b_rows // 128], F32, kind="ExternalOutput")
    with tile.TileContext(nc) as tc:
        build_kwta(tc, out.ap(), brow.ap(), x.ap(), b_rows)
    split_multi_waits(nc)
    return nc


_NC_CACHE = {}


def _get_nc(b_rows):
    if b_rows not in _NC_CACHE:
        _NC_CACHE[b_rows] = _build_module(b_rows)
    return _NC_CACHE[b_rows]


def _host_row_fix(xrow):
    h = np.maximum(xrow, 0.0)
    idx = np.argsort(-h, kind="stable")[:K]
    mask = np.zeros(E, dtype=bool)
    mask[idx] = True
    mask &= xrow > 0
    mask[idx[0]] = True
    return mask


def _host_full(x):
    h = np.maximum(x, 0.0)
    part = np.partition(h, E - K, axis=1)
    t = part[:, E - K:E - K + 1]
    out = (h >= t).astype(np.float32)
    bad = np.flatnonzero(out.sum(axis=1) != float(K))
    for r in bad:
        out[r] = _host_row_fix(x[r]).astype(np.float32)
    return out


def _resolve_boundary(x, vmask, b):
    """Complete the device's ternary mask into the exact top-K mask.

    x: [B, E] f32. vmask: [B, E] u8 Sign output (1 / 0 = winner, 255 =
    below edge(b+1)). b: [B] bucket of the K-th value per row.
    """
    B = x.shape[0]
    win = vmask != 255
    nw = win.sum(axis=1).astype(np.int64)
    m = K - nw

    h16 = (x.view(np.uint32) >> 16).astype(np.int32)
    cand = (h16 == b[:, None].astype(np.int32)) & ~win
    ncand = cand.sum(axis=1).astype(np.int64)

    ok = (m >= 0) & (m <= ncand)
    rows_fix = np.flatnonzero(~ok)

    sel = np.flatnonzero(ok & (m > 0))
    if sel.size:
        # pad each selected row's candidates into a fixed-width matrix and
        # rank with a stable descending sort (ties -> lower column index)
        crows, ccols = np.nonzero(cand[sel])
        counts = ncand[sel]
        maxc = int(counts.max())
        starts = np.concatenate([[0], np.cumsum(counts)[:-1]])
        slot = np.arange(crows.size) - starts[crows]
        padv = np.full((sel.size, maxc), -np.inf, dtype=np.float32)
        padc = np.zeros((sel.size, maxc), dtype=np.int64)
        padv[crows, slot] = x[sel[crows], ccols]
        padc[crows, slot] = ccols
        order = np.argsort(-padv, axis=1, kind="stable")
        take = np.arange(maxc)[None, :] < m[sel][:, None]
        prow, pslot = np.nonzero(take)
        chosen_col = padc[prow, order[prow, pslot]]
        win[sel[prow], chosen_col] = True

    out = win
    # exactness guard: any row without exactly K winners gets the exact path
    bad = np.flatnonzero(out.sum(axis=1) != K)
    for r in np.union1d(rows_fix, bad):
        out[r] = _host_row_fix(x[r])
    return out.astype(np.float32)


def kernel(x: np.ndarray) -> np.ndarray:
    assert x.shape == (B_FULL, E) and x.dtype == np.float32
    try:
        nc = _get_nc(B_CORE)
    except Exception:
        return _host_full(x)
    in_maps = [
        {"x": np.ascontiguousarray(x[i * B_CORE:(i + 1) * B_CORE])}
        for i in range(N_CORES)
    ]
    try:
        res = run_bass_kernel_spmd(nc, in_maps, list(range(N_CORES)))
        vmask = np.concatenate(
            [np.asarray(r["out"]) for r in res.results], axis=0
        )
        # brow[p, ti] holds bdfor core row ti*128 + p
        b = np.concatenate(
            [np.asarray(r["brow"]).T.reshape(-1) for r in res.results], axis=0
        )
    except Exception:
        return _host_full(x)
    return _resolve_boundary(x, vmask, b)


# revision 18
# speedup vs baseline: 1.0250x; 1.0006x over previous
"""KWinnersTakeAll top-k mask (K=410 per row of 8192, relu+mask) for TRN2.

Device finds, per 128-row tile (rows in partitions), each row's 16-bit-prefix
bucket b of the K-th largest value, then emits a ternary mask plus a tiny
per-row side channel; the host resolves only the ~6-7 boundary-bucket
elements per row exactly.

  1. h16 = top-16 bits of each f32 (ACT-engine strided u16 copy). As int16,
     positive floats map to [0, 32767] order-isomorphically.
  2. Newton + bisect on the DVE locates b = the largest 16-bit value with
     G(b) = #{h16 >= b} >= K: one count at V0, a quadratic bucket estimate,
     then a [4,2,1] bisect from round(bhat)-4. Each count is a single i16
     tensor_scalar + accum pass (4x DVE mode, ~2.2us).
  3. ACT emits mask = Sign(edge(b+1) - x) as u8: 0 = winner (x >= edge;
     the u8 conversion saturates Sign's -1 to 0, and x == edge gives 0
     directly), 1 = below the edge. Exactly-rounded f32 subtract makes the
     class boundary exact.
  4. Per-row b values collect into a [128, 8] tile, DMA'd out once.
  5. Host: winners = (mask == 0); m = K - winners_per_row; candidates =
     elements whose top-16 bits == b; add the top-m candidates by exact f32
     value (ties -> lower index, matching the reference). Rows where the
     bracket missed (Newton tail) fall back to an exact host recompute.

Per-tile engine budget: DMA 14.6us (in 11.7 + out 2.9, the bottleneck),
ACT ~14us (h16 copy + Sign mask), DVE ~10us (4 count passes + smalls),
so the kernel runs at the per-core HBM roofline.

The module is post-processed by split_multi_waits: this toolchain's walrus
allows a single sync wait per TPB instruction, so extra semaphore waits are
hoisted onto same-engine NoOps.

Sharding: pure data parallel, 1024 rows per core across 8 cores.
"""

import numpy as np

import concourse.bass as bass
import concourse.mybir as mybir
import concourse.tile as tile
from concourse.bass_utils import run_bass_kernel_spmd


F32 = mybir.dt.float32
U16 = mybir.dt.uint16
I16 = mybir.dt.int16
I32 = mybir.dt.int32
U8 = mybir.dt.uint8
Alu = mybir.AluOpType
Act = mybir.ActivationFunctionType


def split_multi_waits(nc, max_waits: int = 1) -> int:
    """Rewrite every instruction carrying more than ``max_waits`` sem waits.
    Returns the number of instructions split."""
    n_split = 0
    fn = nc.m.functions[0]
    for bb in fn.blocks:
        insts = list(bb.instructions)
        out = []
        changed = False
        for inst in insts:
            si = inst.sync_info
            waits = list(si.on_wait) if si else []
            if len(waits) > max_waits:
                n_split += 1
                changed = True
                for i, w in enumerate(waits[:-max_waits]):
                    nop = mybir.InstNoOp(
                        name=f"{inst.name}-waitsplit-{i}", ins=[], outs=[]
                    )
                    nop.engine = inst.engine
                    nop.sync_info = mybir.SyncInfo(on_wait=[w], on_update=[])
                    out.append(nop)
                inst.sync_info = mybir.SyncInfo(
                    on_wait=waits[-max_waits:], on_update=list(si.on_update)
                )
            out.append(inst)
        if changed:
            bb.instructions = out
    return n_split


B_FULL = 8192
E = 8192
N_CORES = 8
B_CORE = B_FULL // N_CORES
K = 410                  # ceil(0.05 * 8192)
# Newton start: one count at V0, then quadratic bucket estimate and a short
# [4,2,1] bisect from bhat-4. Estimator fit on N(0,1) rows; errors beyond
# [-4,+3] fall to the host guard.
V0 = 16338               # hi16 of ~1.645 (the asymptotic 95% quantile)
NEWT_C0 = 16337.5789     # bhat = C0 + C1*d + C2*d^2, d = count(h16>=V0) - K
NEWT_C1 = 0.14814322
NEWT_C2 = -4.5165e-05
NEWT_STEPS = [4, 2, 1]
NEWT_BACK = 4.0          # bisect lo = round(bhat) - NEWT_BACK


def build_kwta(tc, out_ap, brow_ap, in_ap, b_rows):
    nc = tc.nc
    n_tiles = b_rows // 128

    with (
        tc.tile_pool(name="pio", bufs=3) as pio,
        tc.tile_pool(name="pmask", bufs=1) as pmask,
        tc.tile_pool(name="ph", bufs=2) as ph,
        tc.tile_pool(name="ppred", bufs=1) as ppred,
        tc.tile_pool(name="psmall", bufs=4) as psmall,
        tc.tile_pool(name="pbrow", bufs=1) as pbrow,
    ):
        # one half-width scratch for every count pass's elementwise output
        # (only the accum matters); DVE executes in order so reuse is
        # hazard-free. Counts run as two half-row passes whose accums add.
        H = E // 2
        pred = ppred.tile([128, H], I16)
        ball = pbrow.tile([128, n_tiles], F32)
        masks = []
        Q = E // 4

        def count_ge(h16, scalar, tag):
            """cnt[p] = #{h16[p, :] >= scalar} via two half-row 4x passes."""
            ca = psmall.tile([128, 1], F32, tag=f"{tag}_a", name=f"{tag}_a")
            cb = psmall.tile([128, 1], F32, tag=f"{tag}_b", name=f"{tag}_b")
            nc.vector.tensor_scalar(
                pred[:], h16[:, :H].bitcast(I16), scalar, 0.0, Alu.is_ge,
                Alu.add, accum_out=ca[:],
            )
            nc.vector.tensor_scalar(
                pred[:], h16[:, H:].bitcast(I16), scalar, 0.0, Alu.is_ge,
                Alu.add, accum_out=cb[:],
            )
            cnt = psmall.tile([128, 1], F32, tag=f"{tag}_s", name=f"{tag}_s")
            nc.vector.tensor_tensor(cnt[:], ca[:], cb[:], Alu.add)
            return cnt

        for ti in range(n_tiles):
            rows = slice(ti * 128, (ti + 1) * 128)

            xt = pio.tile([128, E], F32, tag="xt")
            h16 = ph.tile([128, E], U16, tag="h16")
            xu = xt[:].bitcast(U16).rearrange("p (n two) -> p n two", two=2)
            # quarter-split the load so the DVE h16 copy streams behind the
            # DMA instead of waiting for the full 4 MiB transfer
            for qi in range(4):
                nc.sync.dma_start(
                    xt[:, qi * Q:(qi + 1) * Q], in_ap[rows, qi * Q:(qi + 1) * Q]
                )
                # h16 extraction rides the otherwise-idle GPSIMD engine,
                # streaming a quarter behind the DMA
                nc.gpsimd.tensor_copy(
                    h16[:, qi * Q:(qi + 1) * Q], xu[:, qi * Q:(qi + 1) * Q, 1:2]
                )

            # --- Newton start: one count at V0, quadratic bucket estimate --
            # NB: accum_out only accumulates with op1=add (op1=mult silently
            # yields 0 on HW).
            cnt0 = count_ge(h16, float(V0), "cnt0")
            dd = psmall.tile([128, 1], F32, tag="dd")
            nc.vector.tensor_scalar(dd[:], cnt0[:], -float(K), None, Alu.add)
            t1 = psmall.tile([128, 1], F32, tag="t1")
            nc.vector.tensor_scalar(t1[:], dd[:], NEWT_C2, NEWT_C1, Alu.mult, Alu.add)
            t2 = psmall.tile([128, 1], F32, tag="t2")
            nc.vector.tensor_tensor(t2[:], t1[:], dd[:], Alu.mult)
            ri = psmall.tile([128, 1], I32, tag="ri")  # round(bhat)
            nc.vector.tensor_scalar(ri[:], t2[:], NEWT_C0, None, Alu.add)
            vf = psmall.tile([128, 1], F32, tag="vf_e")
            nc.vector.tensor_scalar(vf[:], ri[:], -NEWT_BACK, None, Alu.add)

            # --- bisect: largest b in [lo, lo+8) with G(b) >= K ------------
            for it, s in enumerate(NEWT_STEPS):
                vtest = psmall.tile([128, 1], F32, tag="vtest")
                nc.vector.tensor_scalar(vtest[:], vf[:], float(s), None, Alu.add)
                cnt = count_ge(h16, vtest[:], f"cnt{it % 2}")
                # lo += s if cnt >= K
                ges = psmall.tile([128, 1], F32, tag="ges")
                nc.vector.tensor_scalar(
                    ges[:], cnt[:], float(K) - 0.5, float(s), Alu.is_ge, Alu.mult
                )
                vf2 = psmall.tile(
                    [128, 1], F32, tag=("vf_o" if it % 2 == 0 else "vf_e")
                )
                nc.vector.tensor_tensor(vf2[:], ges[:], vf[:], Alu.add)
                vf = vf2

            # record b for the host side channel
            nc.vector.tensor_copy(ball[:, ti:ti + 1], vf[:])

            # --- binary mask: Sign(edge(b+1) - x) --------------------------
            # edge(b+1) = bitcast((b+1) << 16); (b+1)*65536 is exact in f32
            # (15 significant bits + 16 trailing zeros), converted to i32.
            vb1 = psmall.tile([128, 1], F32, tag="vb1")
            nc.vector.tensor_scalar(vb1[:], vf[:], 1.0, None, Alu.add)
            ei = psmall.tile([128, 1], I32, tag="ei")
            nc.vector.tensor_scalar(ei[:], vb1[:], 65536.0, None, Alu.mult)
            # Sign(-x + edge) with u8 output: losers (x < edge) -> +1;
            # strict winners -> -1, which the u8 conversion SATURATES to 0;
            # x == edge exactly -> 0. So class 0 is exactly the winner set
            # {x >= edge}, with the exactly-rounded f32 subtract making the
            # boundary exact. (A Sign(x - edge) encoding would fold losers
            # and the x == edge winner into one class, u8(-1) == u8(0).)
            mask = pmask.tile([128, E], U8, tag=f"mask{ti}")
            masks.append((mask, rows, ei, xt))
            if ti < n_tiles - 1:
                nc.scalar.activation(
                    mask[:], xt[:], Act.Sign, bias=ei[:].bitcast(F32), scale=-1.0
                )

        # Output DMAs are deferred behind all input loads so the DMA engines
        # stream the 32 MiB of loads back-to-back; the ~23 us of mask stores
        # then overlap the last tiles' compute, and the last tile's mask
        # lands just as the store stream drains.
        for ti in range(n_tiles - 1):
            m_, r_, _, _ = masks[ti]
            nc.sync.dma_start(out_ap[r_, :], m_[:])
        nc.sync.dma_start(brow_ap[:, :], ball[:])
        mask, rows, ei, xt = masks[n_tiles - 1]
        for qi in range(4):
            sl = slice(qi * Q, (qi + 1) * Q)
            nc.scalar.activation(
                mask[:, sl], xt[:, sl], Act.Sign, bias=ei[:].bitcast(F32),
                scale=-1.0,
            )
            nc.sync.dma_start(out_ap[rows, sl], mask[:, sl])


def _build_module(b_rows):
    nc = bass.Bass("TRN2", target_bir_lowering=False, debug=False)
    x = nc.dram_tensor("x", [b_rows, E], F32, kind="ExternalInput")
    out = nc.dram_tensor("out", [b_rows, E], U8, kind="ExternalOutput")
    brow = nc.dram_tensor("brow", [128, b_rows // 128], F32, kind="ExternalOutput")
    with tile.TileContext(nc) as tc:
        build_kwta(tc, out.ap(), brow.ap(), x.ap(), b_rows)
    split_multi_waits(nc)
    return nc


_NC_CACHE = {}


def _get_nc(b_rows):
    if b_rows not in _NC_CACHE:
        _NC_CACHE[b_rows] = _build_module(b_rows)
    return _NC_CACHE[b_rows]


def _host_row_fix(xrow):
    h = np.maximum(xrow, 0.0)
    idx = np.argsort(-h, kind="stable")[:K]
    mask = np.zeros(E, dtype=bool)
    mask[idx] = True
    mask &= xrow > 0
    mask[idx[0]] = True
    return mask


def _host_full(x):
    h = np.maximum(x, 0.0)
    part = np.partition(h, E - K, axis=1)
    t = part[:, E - K:E - K + 1]
    out = (h >= t).astype(np.float32)
    bad = np.flatnonzero(out.sum(axis=1) != float(K))
    for r in bad:
        out[r] = _host_row_fix(x[r]).astype(np.float32)
    return out


def _resolve_boundary(x, vmask, b):
    """Complete the device's ternary mask into the exact top-K mask.

    x: [B, E] f32. vmask: [B, E] u8 Sign(edge - x) output (0 = winner,
    i.e. x >= edge(b+1); 1 = below the edge). b: [B] bucket of the K-th
    value per row.
    """
    B = x.shape[0]
    win = vmask == 0
    nw = win.sum(axis=1).astype(np.int64)
    m = K - nw

    h16 = (x.view(np.uint32) >> 16).astype(np.int32)
    cand = (h16 == b[:, None].astype(np.int32)) & ~win
    ncand = cand.sum(axis=1).astype(np.int64)

    ok = (m >= 0) & (m <= ncand)
    rows_fix = np.flatnonzero(~ok)

    sel = np.flatnonzero(ok & (m > 0))
    if sel.size:
        # pad each selected row's candidates into a fixed-width matrix and
        # rank with a stable descending sort (ties -> lower column index)
        crows, ccols = np.nonzero(cand[sel])
        counts = ncand[sel]
        maxc = int(counts.max())
        starts = np.concatenate([[0], np.cumsum(counts)[:-1]])
        slot = np.arange(crows.size) - starts[crows]
        padv = np.full((sel.size, maxc), -np.inf, dtype=np.float32)
        padc = np.zeros((sel.size, maxc), dtype=np.int64)
        padv[crows, slot] = x[sel[crows], ccols]
        padc[crows, slot] = ccols
        order = np.argsort(-padv, axis=1, kind="stable")
        take = np.arange(maxc)[None, :] < m[sel][:, None]
        prow, pslot = np.nonzero(take)
        chosen_col = padc[prow, order[prow, pslot]]
        win[sel[prow], chosen_col] = True

    out = win
    # exactness guard: any row without exactly K winners gets the exact path
    bad = np.flatnonzero(out.sum(axis=1) != K)
    for r in np.union1d(rows_fix, bad):
        out[r] = _host_row_fix(x[r])
    return out.astype(np.float32)


def kernel(x: np.ndarray) -> np.ndarray:
    assert x.shape == (B_FULL, E) and x.dtype == np.float32
    try:
        nc = _get_nc(B_CORE)
    except Exception:
        return _host_full(x)
    in_maps = [
        {"x": np.ascontiguousarray(x[i * B_CORE:(i + 1) * B_CORE])}
        for i in range(N_CORES)
    ]
    try:
        res = run_bass_kernel_spmd(nc, in_maps, list(range(N_CORES)))
        vmask = np.concatenate(
            [np.asarray(r["out"]) for r in res.results], axis=0
        )
        # brow[p, ti] holds bdfor core row ti*128 + p
        b = np.concatenate(
            [np.asarray(r["brow"]).T.reshape(-1) for r in res.results], axis=0
        )
    except Exception:
        return _host_full(x)
    return _resolve_boundary(x, vmask, b)
